# revision 1
# baseline (speedup 1.0000x reference)
"""Bahdanau-style attention kernel for Trainium2, data-parallel over batch on 8 cores.

reference:
    q = query @ W1 + b1                              [B, H]
    lk = key @ W2 + b2                               [B, S, H]
    score = softmax(tanh(q[:,None]+lk) @ v + bv)     [B, S]   (bv drops: softmax shift-invariant)
    context = score @ key                            [B, D]
Outputs (context, score).

Sharding: batch B=32 split 4-per-core across 8 NeuronCores; weights replicated.

Per-core schedule (all matmuls in float32r, ~2e-4 rel rmse):
  for b in 0..3:
    phase1(b): stream key[b] in s-tiles; PE-transpose each tile to [d,s];
               lk_T = W2.T @ key_T (stationary W2, f32r); tanh(+q+b1+b2) fused
               on ScalarE with per-partition bias; score = v.T @ tanh via PE.
    softmax(b) on the [1, S] score row; store probs.
    phase2(b): stream key[b] again (SWDGE f32->f32r cast); ctx_T[d,1] += key.T @ p
               (key chunk stationary) -- overlaps phase1(b+1) compute.
"""

import numpy as np

import concourse.bacc as bacc
import concourse.mybir as mybir
from concourse.tile import TileContext
from concourse.masks import make_identity

f32 = mybir.dt.float32
f32r = mybir.dt.float32r
P = 128
Tanh = mybir.ActivationFunctionType.Tanh
Exp = mybir.ActivationFunctionType.Exp
Identity = mybir.ActivationFunctionType.Identity
AxX = mybir.AxisListType.X
Max = mybir.AluOpType.max


def build_nc(B_L=4, S=2048, D=2048, H=1024, Q=1024, S_T=256, swdge_cast=True):
    SC = S_T // P      # s sub-chunks per s-tile
    ST_N = S // S_T    # number of s-tiles
    DC = D // P        # contraction chunks for lin2 (d)
    HC = H // P        # h chunks
    QC = Q // P        # q contraction chunks
    SK = S // P        # 128-wide s chunks (phase 2)
    assert B_L <= 4

    nc = bacc.Bacc(None, target_bir_lowering=False)
    query = nc.dram_tensor("query", [B_L, Q], f32, kind="ExternalInput")
    key = nc.dram_tensor("key", [B_L, S, D], f32, kind="ExternalInput")
    W1 = nc.dram_tensor("W1", [Q, H], f32, kind="ExternalInput")
    b1 = nc.dram_tensor("b1", [H], f32, kind="ExternalInput")
    W2 = nc.dram_tensor("W2", [D, H], f32, kind="ExternalInput")
    b2 = nc.dram_tensor("b2", [H], f32, kind="ExternalInput")
    v = nc.dram_tensor("v", [H], f32, kind="ExternalInput")
    ctx_out = nc.dram_tensor("ctx_out", [B_L, D], f32, kind="ExternalOutput")
    score_out = nc.dram_tensor("score_out", [B_L, S], f32, kind="ExternalOutput")

    with TileContext(nc) as tc:
        with (
            tc.tile_pool(name="const", bufs=1) as const_pool,
            tc.tile_pool(name="w2p", bufs=1) as w2_pool,
            tc.tile_pool(name="knat", bufs=3) as knat_pool,
            tc.tile_pool(name="kt", bufs=2) as kt_pool,
            tc.tile_pool(name="th", bufs=2) as th_pool,
            tc.tile_pool(name="sm", bufs=1) as sm_pool,
            tc.tile_pool(name="stage", bufs=2) as stage_pool,
            tc.tile_pool(name="ps", bufs=1, space="PSUM") as ps,
        ):
            # ---------------- prologue: constants, weights, q/bias ----------------
            ident = const_pool.tile([P, P], f32)
            make_identity(nc, ident)
            ident_r = const_pool.tile([P, P], f32r)
            nc.vector.tensor_copy(ident_r[:], ident[:])

            # prefetch the first key tile ahead of all weight traffic
            preknat = {}
            def load_knat(bb, st):
                t = knat_pool.tile([P, SC, D], f32r, name="knat")
                nc.gpsimd.dma_start(
                    t[:],
                    key[bb, st * S_T:(st + 1) * S_T, :].rearrange(
                        "(sc p) d -> p sc d", p=P
                    ),
                )
                return t
            preknat[0] = load_knat(0, 0)

            # query / b1 / b2 / v: natural-layout loads + PE transpose
            qstage = const_pool.tile([B_L, Q], f32)
            nc.sync.dma_start(qstage[:], query[:, :])
            bstage = const_pool.tile([4, H], f32)
            nc.vector.memset(bstage[:], 0.0)
            nc.sync.dma_start(bstage[0:1, :], b1[None, :])
            nc.sync.dma_start(bstage[1:2, :], b2[None, :])
            nc.sync.dma_start(bstage[2:3, :], v[None, :])

            qT_sb = const_pool.tile([P, QC, B_L], f32)
            q_tp = ps.tile([P, QC, B_L], f32, name="q_tp", tag="scpt", bufs=2)
            for k in range(QC):
                nc.tensor.transpose(
                    q_tp[:, k, :], qstage[:, k * P:(k + 1) * P], ident[:B_L, :B_L]
                )
            nc.vector.tensor_copy(qT_sb[:], q_tp[:])

            # hoist the first key tile's transposes: fills PE while the
            # bias/W1 DMAs are still in flight (PE streams execute in order)
            pre_kt = kt_pool.tile([P, DC, S_T], f32r, name="kt")
            for dc2 in range(DC // 2):
                tp_ps = ps.tile([P, 2, S_T], f32r, name="tp_ps", bufs=2)
                for j in range(2):
                    for sc in range(SC):
                        nc.tensor.transpose(
                            tp_ps[:, j, sc * P:(sc + 1) * P],
                            preknat[0][:, sc, (2 * dc2 + j) * P:(2 * dc2 + j + 1) * P],
                            ident_r[:],
                        )
                nc.vector.tensor_copy(pre_kt[:, 2 * dc2:2 * dc2 + 2, :], tp_ps[:])

            bt_ps = ps.tile([P, HC, 4], f32, name="bt_ps", tag="scpt", bufs=2)
            for m in range(HC):
                nc.tensor.transpose(
                    bt_ps[:, m, :], bstage[:, m * P:(m + 1) * P], ident[:4, :4]
                )
            bt_sb = const_pool.tile([P, HC, 3], f32)
            nc.vector.tensor_copy(bt_sb[:], bt_ps[:, :, 0:3])
            b12_sb = const_pool.tile([P, HC], f32)
            nc.vector.tensor_tensor(
                b12_sb[:], bt_sb[:, :, 0], bt_sb[:, :, 1], op=mybir.AluOpType.add
            )
            v_r = const_pool.tile([P, HC], f32r)
            nc.vector.tensor_copy(v_r[:], bt_sb[:, :, 2])

            # bias[h, b] = query@W1 + b1 + b2 (per-batch per-partition tanh bias)
            bias_sb = const_pool.tile([P, HC, B_L], f32)
            q_ps = ps.tile([P, HC, B_L], f32, name="q_ps", tag="scpt", bufs=2)
            QH = QC // 2
            w1a = knat_pool.tile([P, QH, H], f32, name="knat")
            nc.sync.dma_start(
                w1a[:], W1[0:QH * P, :].rearrange("(k p) h -> p k h", p=P)
            )
            w1b = knat_pool.tile([P, QH, H], f32, name="knat")
            nc.sync.dma_start(
                w1b[:], W1[QH * P:, :].rearrange("(k p) h -> p k h", p=P)
            )
            for k in range(QC):
                w1c = (w1a if k < QH else w1b)[:, k % QH, :]
                for m in range(HC):
                    nc.tensor.matmul(
                        q_ps[:, m, :],
                        w1c[:, m * P:(m + 1) * P],
                        qT_sb[:, k, :],
                        start=(k == 0 and m == 0),
                        stop=(k == QC - 1 and m == HC - 1),
                        skip_group_check=True,
                    )
            for m in range(HC):
                nc.scalar.activation(
                    bias_sb[:, m, :], q_ps[:, m, :], Identity,
                    bias=b12_sb[:, m:m + 1],
                )

            # W2 -> f32r directly via SWDGE cast-on-DMA
            w2_r = w2_pool.tile([P, DC, H], f32r)
            W2G = min(4, DC)
            for g0 in range(0, DC, W2G):
                nc.gpsimd.dma_start(
                    w2_r[:, g0:g0 + W2G, :],
                    W2[g0 * P:(g0 + W2G) * P, :].rearrange(
                        "(dc p) h -> p dc h", p=P
                    ),
                )
            for st in range(1, min(3, ST_N)):
                preknat[st] = load_knat(0, st)

            # ------- per-batch pipeline: online softmax + context (single key pass)
            NEG_INF = -1.0e30
            Mult = mybir.AluOpType.mult
            Add = mybir.AluOpType.add
            DQ = D // 512
            for b in range(B_L):
                raw_b = sm_pool.tile([1, S], f32, name="raw_b", bufs=1)
                ctx_acc = sm_pool.tile([1, D], f32, name="ctx_acc", bufs=2)
                # st0 cols: 0=m_run, 1=m_st, 2=-m_new, 3=scale, 4=sum_st, 5=sum_acc, 6=ri
                st0 = sm_pool.tile([1, 8], f32, name="st0", bufs=2)
                nc.vector.memset(st0[:], 0.0)
                nc.vector.memset(st0[0:1, 0:1], NEG_INF)
                nc.vector.memset(ctx_acc[:], 0.0)
                for st in range(ST_N):
                    if b == 0 and st in preknat:
                        knat = preknat.pop(st)
                    else:
                        knat = load_knat(b, st)
                    # transpose to [d, s] (f32r), 128x128 blocks via PE
                    if b == 0 and st == 0:
                        kt = pre_kt
                    else:
                        kt = kt_pool.tile([P, DC, S_T], f32r, name="kt")
                        for dc2 in range(DC // 2):
                            tp_ps = ps.tile([P, 2, S_T], f32r, name="tp_ps", bufs=2)
                            for j in range(2):
                                for sc in range(SC):
                                    nc.tensor.transpose(
                                        tp_ps[:, j, sc * P:(sc + 1) * P],
                                        knat[:, sc, (2 * dc2 + j) * P:(2 * dc2 + j + 1) * P],
                                        ident_r[:],
                                    )
                            nc.vector.tensor_copy(kt[:, 2 * dc2:2 * dc2 + 2, :], tp_ps[:])
                    # lin2 + tanh: th[h, s] = tanh(q_b + b1 + b2 + W2.T kt)
                    th = th_pool.tile([P, HC, S_T], f32r, name="th")
                    for m in range(HC):
                        lk_ps = ps.tile([P, S_T], f32, name="lk_ps", bufs=2)
                        for dc in range(DC):
                            nc.tensor.matmul(
                                lk_ps[:],
                                w2_r[:, dc, m * P:(m + 1) * P],
                                kt[:, dc, :],
                                start=(dc == 0),
                                stop=(dc == DC - 1),
                            )
                        nc.scalar.activation(
                            th[:, m, :], lk_ps[:], Tanh,
                            bias=bias_sb[:, m, b:b + 1],
                        )
                    # score[s] = sum_h v[h] th[h, s]
                    sc_ps = ps.tile([1, S_T], f32, name="sc_ps", tag="scpt", bufs=2)
                    for m in range(HC):
                        nc.tensor.matmul(
                            sc_ps[:],
                            v_r[:, m:m + 1],
                            th[:, m, :],
                            start=(m == 0),
                            stop=(m == HC - 1),
                        )
                    # raw scores kept for the final normalized score output
                    nc.scalar.copy(raw_b[0:1, st * S_T:(st + 1) * S_T], sc_ps[:])
                    # online-softmax stats (all single-lane at partition 0)
                    nc.vector.tensor_reduce(
                        st0[0:1, 1:2], sc_ps[:], axis=AxX, op=Max
                    )
                    nc.vector.tensor_reduce(
                        st0[0:1, 2:3], st0[0:1, 0:2], axis=AxX, op=Max, negate=True
                    )
                    nc.scalar.activation(
                        st0[0:1, 3:4], st0[0:1, 0:1], Exp, bias=st0[0:1, 2:3]
                    )
                    nc.scalar.mul(st0[0:1, 0:1], st0[0:1, 2:3], -1.0)
                    exp4 = sm_pool.tile([4, S_T], f32, name="exp4", bufs=2)
                    nc.vector.memset(exp4[:], 0.0)
                    nc.scalar.activation(
                        exp4[0:1, :], sc_ps[:], Exp, bias=st0[0:1, 2:3],
                        accum_out=st0[0:1, 4:5],
                    )
                    nc.vector.scalar_tensor_tensor(
                        st0[0:1, 5:6], st0[0:1, 5:6], st0[0:1, 3:4],
                        st0[0:1, 4:5], op0=Mult, op1=Add,
                    )
                    # exp row -> [s, 1] f32r columns via PE transpose
                    ptc = sm_pool.tile([P, SC], f32r, name="ptc", bufs=2)
                    for sc in range(SC):
                        ptp = ps.tile([P, 4], f32, name="ptp", tag="scpt", bufs=2)
                        nc.tensor.transpose(
                            ptp[:], exp4[:, sc * P:(sc + 1) * P], ident[:4, :4]
                        )
                        nc.vector.tensor_copy(ptc[:, sc:sc + 1], ptp[:, 0:1])
                    # context partial over this s-tile (moving key, M=1 N=512),
                    # then rescale-accumulate into SBUF
                    for qd in range(DQ):
                        cpp = ps.tile([1, 512], f32, name="cpp", tag="ctxp", bufs=2)
                        for sc in range(SC):
                            nc.tensor.matmul(
                                cpp[:],
                                ptc[:, sc:sc + 1],
                                knat[:, sc, qd * 512:(qd + 1) * 512],
                                start=(sc == 0),
                                stop=(sc == SC - 1),
                            )
                        nc.vector.scalar_tensor_tensor(
                            ctx_acc[0:1, qd * 512:(qd + 1) * 512],
                            ctx_acc[0:1, qd * 512:(qd + 1) * 512],
                            st0[0:1, 3:4], cpp[:], op0=Mult, op1=Add,
                        )
                # ---- finalize batch b
                nc.vector.reciprocal(st0[0:1, 6:7], st0[0:1, 5:6])
                nc.scalar.activation(
                    raw_b[:], raw_b[:], Exp, bias=st0[0:1, 2:3]
                )
                nc.vector.tensor_scalar_mul(raw_b[:], raw_b[:], st0[0:1, 6:7])
                nc.sync.dma_start(score_out[b:b + 1, :], raw_b[:])
                nc.vector.tensor_scalar_mul(ctx_acc[:], ctx_acc[:], st0[0:1, 6:7])
                nc.sync.dma_start(ctx_out[b:b + 1, :], ctx_acc[:])

    nc.compile()
    return nc


_NC_CACHE = {}


def _get_nc():
    if "full" not in _NC_CACHE:
        _NC_CACHE["full"] = build_nc()
    return _NC_CACHE["full"]


def kernel(**inputs):
    from concourse.bass_utils import run_bass_kernel_spmd

    query = np.ascontiguousarray(inputs["query"], dtype=np.float32)
    key = np.ascontiguousarray(inputs["key"], dtype=np.float32)
    W1 = np.ascontiguousarray(inputs["W1"], dtype=np.float32)
    b1 = np.ascontiguousarray(inputs["b1"], dtype=np.float32)
    W2 = np.ascontiguousarray(inputs["W2"], dtype=np.float32)
    b2 = np.ascontiguousarray(inputs["b2"], dtype=np.float32)
    v = np.ascontiguousarray(inputs["v"], dtype=np.float32)
    # bv is softmax-shift-invariant: it cancels in both outputs.

    B = query.shape[0]
    n_cores = 8
    B_L = B // n_cores

    nc = _get_nc()
    in_maps = []
    for c in range(n_cores):
        sl = slice(c * B_L, (c + 1) * B_L)
        in_maps.append({
            "query": query[sl], "key": key[sl],
            "W1": W1, "b1": b1, "W2": W2, "b2": b2, "v": v,
        })
    res = run_bass_kernel_spmd(nc, in_maps, core_ids=list(range(n_cores)))
    context = np.concatenate([r["ctx_out"] for r in res.results], axis=0)
    score = np.concatenate([r["score_out"] for r in res.results], axis=0)
    return (context.astype(np.float32), score.astype(np.float32))



# revision 6
# speedup vs baseline: 3.4081x; 3.4081x over previous
"""Bahdanau-style attention kernel for Trainium2, data-parallel over batch on 8 cores.

reference:
    q = query @ W1 + b1                              [B, H]
    lk = key @ W2 + b2                               [B, S, H]
    score = softmax(tanh(q[:,None]+lk) @ v + bv)     [B, S]   (bv drops: softmax shift-invariant)
    context = score @ key                            [B, D]
Outputs (context, score).

Sharding: batch B=32 split 4-per-core across 8 NeuronCores; weights replicated.

v2 design (fp8 DoubleRow):
  - host quantizes key -> fp8e4 (RNE, == TRN fp8_exp4 for |x|<240) plus an
    fp8 residual (key - fp8(key)) used to rebuild ~bf16-accurate key for the
    context matmul. W2*64 and v*64 are host-quantized to fp8 (scale avoids
    subnormals; tanh/exp activations compensate with scale=1/64).
  - per s-tile: key8 natural tile -> PE transpose of bf16-viewed fp8 PAIRS
    (bit-exact, probe-validated) -> kt packed [d-pair, s] -> lin2 via fp8
    DoubleRow matmuls (K=256/instr, 0.5 cyc/row) -> tanh (fused bias q+b1+b2,
    fp8 out) -> score via DoubleRow with v8 -> exp (no max shift needed:
    |score| <~ 2, exp cannot overflow) -> probs transposed to columns ->
    context accumulated in PSUM via tiny stationary-key matmuls
    (fp8 hi + fp8 lo residual) x bf16 prob column.
  - softmax normalization applied at batch end (1/sum via DVE reciprocal,
    partition-broadcast for the context scale).
  - software pipeline: transposes run one tile ahead; exp-transpose + context
    of tile t run in the middle of tile t+1's lin2 to hide latencies.
"""

import numpy as np

import concourse.bacc as bacc
import concourse.mybir as mybir
from concourse.tile import TileContext
from concourse.masks import make_identity

f32 = mybir.dt.float32
bf16 = mybir.dt.bfloat16
fp8 = mybir.dt.float8e4
P = 128
DR = mybir.MatmulPerfMode.DoubleRow
Tanh = mybir.ActivationFunctionType.Tanh
Exp = mybir.ActivationFunctionType.Exp
Copy = mybir.ActivationFunctionType.Copy
Identity = mybir.ActivationFunctionType.Identity
AxX = mybir.AxisListType.X
Add = mybir.AluOpType.add

B_L, S, D, H, Q = 4, 2048, 2048, 1024, 1024
S_T = 256                 # s-tile size
SC = S_T // P             # 2 s-chunks per tile
ST_N = S // S_T           # 8 s-tiles per batch
NT = B_L * ST_N           # 32 tiles total
DC2 = D // 256            # 8 contraction chunks (256 d each) for DoubleRow
HC = H // P               # 8 h-blocks
QC = Q // P               # 8 q-chunks
DQ = D // P               # 16 d-blocks for context
VSCALE = 64.0             # host scale on W2 and v (fp8 subnormal avoidance)


def build_nc():
    nc = bacc.Bacc(None, target_bir_lowering=False)
    key8 = nc.dram_tensor("key8", [B_L, S, D], fp8, kind="ExternalInput")
    key8lo = nc.dram_tensor("key8lo", [B_L, S, D], fp8, kind="ExternalInput")
    w28 = nc.dram_tensor("w28", [P, DC2, 2, H], fp8, kind="ExternalInput")
    w1b = nc.dram_tensor("w1b", [P, QC, H], bf16, kind="ExternalInput")
    qt16 = nc.dram_tensor("qt16", [P, QC, B_L], bf16, kind="ExternalInput")
    b12r = nc.dram_tensor("b12r", [P, HC], f32, kind="ExternalInput")
    vs8 = nc.dram_tensor("vs8", [P, HC // 2, 2, 16], fp8, kind="ExternalInput")
    ctx_out = nc.dram_tensor("ctx_out", [B_L, D], f32, kind="ExternalOutput")
    score_out = nc.dram_tensor("score_out", [B_L, S], f32, kind="ExternalOutput")

    with TileContext(nc) as tc:
        with (
            tc.tile_pool(name="const", bufs=1) as const_pool,
            tc.tile_pool(name="k8p", bufs=4) as k8_pool,
            tc.tile_pool(name="k8lop", bufs=3) as k8lo_pool,
            tc.tile_pool(name="ktp", bufs=2) as kt_pool,
            tc.tile_pool(name="thp", bufs=2) as th_pool,
            tc.tile_pool(name="pcp", bufs=3) as pc_pool,
            tc.tile_pool(name="smp", bufs=1) as sm_pool,
            tc.tile_pool(name="ps", bufs=1, space="PSUM") as ps,
        ):
            # ---------------- prologue ----------------
            ident = const_pool.tile([P, P], f32)
            make_identity(nc, ident)
            ident16 = const_pool.tile([P, P], bf16)
            nc.vector.tensor_copy(ident16[:], ident[:])

            # small weights first: bias path has the longest dependency chain
            w1s = const_pool.tile([P, QC, H], bf16)
            nc.sync.dma_start(w1s[:], w1b[:, :, :])
            qts = const_pool.tile([P, QC, B_L], bf16)
            nc.sync.dma_start(qts[:], qt16[:, :, :])
            b12s = const_pool.tile([P, HC], f32)
            nc.sync.dma_start(b12s[:], b12r[:, :])
            vss = const_pool.tile([P, HC // 2, 2, 16], fp8)
            nc.sync.dma_start(vss[:], vs8[:, :, :, :])

            # first key tiles ahead of the big weight DMA
            def load_k8(t, pool, src):
                b, st = divmod(t, ST_N)
                tile = pool.tile([P, SC, D], fp8, name=pool.name + "t")
                nc.sync.dma_start(
                    tile[:],
                    src[b, st * S_T:(st + 1) * S_T, :].rearrange(
                        "(sc p) d -> p sc d", p=P
                    ),
                )
                return tile

            k8_t = {0: load_k8(0, k8_pool, key8)}

            w28s = const_pool.tile([P, DC2, 2, H], fp8)
            nc.sync.dma_start(w28s[:], w28[:, :, :, :])

            k8_t[1] = load_k8(1, k8_pool, key8)
            k8lo_t = {0: load_k8(0, k8lo_pool, key8lo)}

            # bias_sb[h, m, b] = (query @ W1)[b, h] + b1[h] + b2[h]
            q_ps = ps.tile([P, HC, B_L], f32, name="q_ps", tag="scpt", bufs=2)
            for m in range(HC):
                for k in range(QC):
                    nc.tensor.matmul(
                        q_ps[:, m, :],
                        w1s[:, k, m * P:(m + 1) * P],
                        qts[:, k, :],
                        start=(k == 0),
                        stop=(k == QC - 1),
                        skip_group_check=True,
                    )
            bias_sb = const_pool.tile([P, HC, B_L], f32)
            for m in range(HC):
                nc.scalar.activation(
                    bias_sb[:, m, :], q_ps[:, m, :], Identity,
                    bias=b12s[:, m:m + 1],
                )

            exp4 = const_pool.tile([4, S_T], f32)
            nc.vector.memset(exp4[:], 0.0)
            # zero operands for the ctx-bank clearing matmul (PSUM start=True
            # marks the whole 2KB bank pending-zero, so the bank must be
            # cleared by ONE matmul covering all columns, never per-column)
            zc16 = const_pool.tile([1, P], bf16)
            nc.vector.memset(zc16[:], 0.0)
            zm16 = const_pool.tile([1, DQ], bf16)
            nc.vector.memset(zm16[:], 0.0)

            # ---------------- pipelined main loop ----------------
            # body(t): dma(t+2); transposes(t+1); lin2(t) m0..3; [exp4tp+ctx](t-1);
            #          lin2(t) m4..7; score(t); exp(t)
            kt_t = {}
            pcol_t = {}
            th_t = {}
            exp_row_b = {}
            sums_b = {}
            ctxps_b = {}

            def transposes(t):
                k16v = k8_t[t][:].bitcast(bf16)  # [P, SC, D//2]
                kt = kt_pool.tile([P, DC2, SC, P], bf16, name="kt")
                for half in range(2):
                    tp16 = ps.tile([P, DC2 // 2, SC, P], bf16, name="tp16", bufs=2)
                    for fq in range(DC2 // 2):
                        fb = half * (DC2 // 2) + fq
                        for sc in range(SC):
                            nc.tensor.transpose(
                                tp16[:, fq, sc, :],
                                k16v[:, sc, fb * P:(fb + 1) * P],
                                ident16[:],
                            )
                    nc.vector.tensor_copy(
                        kt[:, half * (DC2 // 2):(half + 1) * (DC2 // 2)], tp16[:]
                    )
                kt_t[t] = kt

            def lin2(t, m_lo, m_hi):
                b = t // ST_N
                kt = kt_t[t]
                if m_lo == 0:
                    th_t[t] = th_pool.tile([P, HC, S_T], fp8, name="th")
                th = th_t[t]
                for m in range(m_lo, m_hi):
                    lk = ps.tile([P, S_T], f32, name="lk", bufs=2)
                    for c in range(DC2):
                        mv = kt[:, c].bitcast(fp8).rearrange(
                            "p sc (s two) -> p two sc s", two=2
                        )
                        nc.tensor.matmul(
                            lk[:],
                            w28s[:, c, :, m * P:(m + 1) * P],
                            mv,
                            start=(c == 0),
                            stop=(c == DC2 - 1),
                            perf_mode=DR,
                            skip_group_check=True,
                        )
                    nc.scalar.activation(
                        th[:, m, :], lk[:], Tanh,
                        bias=bias_sb[:, m, b:b + 1], scale=1.0 / VSCALE,
                    )

            def score_exp(t):
                b, st = divmod(t, ST_N)
                th = th_t[t]
                sc_ps = ps.tile([1, S_T], f32, name="sc_ps", tag="scpt", bufs=2)
                for m2 in range(HC // 2):
                    nc.tensor.matmul(
                        sc_ps[:],
                        vss[:, m2, :, 0:1],
                        th[:, 2 * m2:2 * m2 + 2, :],
                        start=(m2 == 0),
                        stop=(m2 == HC // 2 - 1),
                        perf_mode=DR,
                        skip_group_check=True,
                    )
                if st == 0:
                    exp_row_b[b] = sm_pool.tile([1, S], f32, name="exp_row", bufs=2)
                    sums_b[b] = sm_pool.tile([1, ST_N], f32, name="sums", bufs=2)
                nc.scalar.activation(
                    exp4[0:1, :], sc_ps[:], Exp, scale=1.0 / VSCALE,
                    accum_out=sums_b[b][0:1, st:st + 1],
                )
                nc.vector.tensor_copy(
                    exp_row_b[b][0:1, st * S_T:(st + 1) * S_T], exp4[0:1, :]
                )

            def exp_cols_ctx(t):
                b, st = divmod(t, ST_N)
                # probs columns (bf16) from exp4 row
                pcol = pc_pool.tile([P, SC], bf16, name="pcol")
                for sc in range(SC):
                    ptp = ps.tile([P, 4], f32, name="ptp", tag="scpt", bufs=2)
                    nc.tensor.transpose(
                        ptp[:], exp4[:, sc * P:(sc + 1) * P], ident[:4, :4]
                    )
                    nc.vector.tensor_copy(pcol[:, sc:sc + 1], ptp[:, 0:1])
                pcol_t[t] = pcol
                # context accumulation: stationary key blocks x prob column
                if st == 0:
                    ctxps_b[b] = ps.tile([P, DQ], f32, name="ctx_ps", bufs=2)
                    nc.tensor.matmul(
                        ctxps_b[b][:], zc16[:], zm16[:],
                        start=True, stop=False, skip_group_check=True,
                    )
                ctx_ps = ctxps_b[b]
                for sc in range(SC):
                    for dq in range(DQ):
                        nc.tensor.matmul(
                            ctx_ps[:, dq:dq + 1],
                            k8_t[t][:, sc, dq * P:(dq + 1) * P],
                            pcol[:, sc:sc + 1],
                            start=False,
                            stop=False,
                            skip_group_check=True,
                        )
                        nc.tensor.matmul(
                            ctx_ps[:, dq:dq + 1],
                            k8lo_t[t][:, sc, dq * P:(dq + 1) * P],
                            pcol[:, sc:sc + 1],
                            start=False,
                            stop=(st == ST_N - 1 and sc == SC - 1 and dq == DQ - 1),
                            skip_group_check=True,
                        )
                del k8_t[t], k8lo_t[t], pcol_t[t]

            def finalize(b):
                tot = sm_pool.tile([1, 2], f32, name="tot", bufs=2)
                nc.vector.tensor_reduce(
                    tot[0:1, 0:1], sums_b[b][:], axis=AxX, op=Add
                )
                nc.vector.reciprocal(tot[0:1, 1:2], tot[0:1, 0:1])
                inv = tot[0:1, 1:2]
                erow = exp_row_b[b]
                nc.vector.tensor_scalar_mul(erow[:], erow[:], inv)
                nc.sync.dma_start(score_out[b:b + 1, :], erow[:])
                inv128 = sm_pool.tile([P, 1], f32, name="inv128", bufs=2)
                nc.gpsimd.partition_broadcast(inv128[:], inv)
                ctx_sb = sm_pool.tile([P, DQ], f32, name="ctx_sb", bufs=2)
                nc.scalar.activation(
                    ctx_sb[:], ctxps_b[b][:], Copy, scale=inv128[:, 0:1]
                )
                crow_ps = ps.tile([DQ, P], f32, name="crow", tag="scpt", bufs=2)
                nc.tensor.transpose(crow_ps[:], ctx_sb[:], ident[:])
                crow = sm_pool.tile([DQ, P], f32, name="crow_sb", bufs=2)
                nc.vector.tensor_copy(crow[:], crow_ps[:])
                nc.sync.dma_start(
                    ctx_out[b].rearrange("(dq p) -> dq p", p=P), crow[:]
                )
                del exp_row_b[b], sums_b[b], ctxps_b[b]

            for t in range(NT + 1):
                if t + 2 <= NT - 1:
                    k8_t[t + 2] = load_k8(t + 2, k8_pool, key8)
                if t + 1 <= NT - 1:
                    k8lo_t[t + 1] = load_k8(t + 1, k8lo_pool, key8lo)
                if t == 0:
                    transposes(0)
                if t + 1 <= NT - 1:
                    transposes(t + 1)
                if t <= NT - 1:
                    lin2(t, 0, HC // 2)
                if t >= 1:
                    exp_cols_ctx(t - 1)
                    if t % ST_N == 0:
                        finalize(t // ST_N - 1)
                if t <= NT - 1:
                    lin2(t, HC // 2, HC)
                    score_exp(t)

    nc.compile()
    return nc


_NC_CACHE = {}


def _get_nc():
    if "full" not in _NC_CACHE:
        _NC_CACHE["full"] = build_nc()
    return _NC_CACHE["full"]


def kernel(**inputs):
    import ml_dtypes
    from concourse.bass_utils import run_bass_kernel_spmd

    E4 = ml_dtypes.float8_e4m3
    BF = ml_dtypes.bfloat16

    query = np.ascontiguousarray(inputs["query"], dtype=np.float32)
    key = np.ascontiguousarray(inputs["key"], dtype=np.float32)
    W1 = np.ascontiguousarray(inputs["W1"], dtype=np.float32)
    b1 = np.ascontiguousarray(inputs["b1"], dtype=np.float32)
    W2 = np.ascontiguousarray(inputs["W2"], dtype=np.float32)
    b2 = np.ascontiguousarray(inputs["b2"], dtype=np.float32)
    v = np.ascontiguousarray(inputs["v"], dtype=np.float32)
    # bv is softmax-shift-invariant: it cancels in both outputs.

    B = query.shape[0]
    n_cores = 8
    bl = B // n_cores

    # host-side quantization (RNE; ml_dtypes float8_e4m3 == TRN fp8_exp4
    # bit-for-bit in our value range)
    key8 = key.astype(E4)
    key8lo = (key - key8.astype(np.float32)).astype(E4)
    w28 = (W2 * VSCALE).astype(np.float32).reshape(DC2, P, 2, H) \
        .transpose(1, 0, 2, 3).astype(E4)
    w28 = np.ascontiguousarray(w28)
    w1b = np.ascontiguousarray(W1.reshape(QC, P, H).transpose(1, 0, 2)).astype(BF)
    b12 = np.ascontiguousarray((b1 + b2).reshape(HC, P).T, dtype=np.float32)
    vs = np.zeros((P, HC // 2, 2, 16), dtype=E4)
    vs[:, :, :, 0] = (v * VSCALE).reshape(HC // 2, 2, P).transpose(2, 0, 1).astype(E4)

    nc = _get_nc()
    in_maps = []
    for c in range(n_cores):
        sl = slice(c * bl, (c + 1) * bl)
        qt = np.ascontiguousarray(
            query[sl].reshape(bl, QC, P).transpose(2, 1, 0)
        ).astype(BF)
        in_maps.append({
            "key8": key8[sl], "key8lo": key8lo[sl],
            "w28": w28, "w1b": w1b, "qt16": qt, "b12r": b12, "vs8": vs,
        })
    res = run_bass_kernel_spmd(nc, in_maps, core_ids=list(range(n_cores)))
    context = np.concatenate([r["ctx_out"] for r in res.results], axis=0)
    score = np.concatenate([r["score_out"] for r in res.results], axis=0)
    return (context.astype(np.float32), score.astype(np.float32))


# revision 7
# speedup vs baseline: 3.7700x; 1.1062x over previous
"""Bahdanau-style attention kernel for Trainium2, data-parallel over batch on 8 cores.

reference:
    q = query @ W1 + b1                              [B, H]
    lk = key @ W2 + b2                               [B, S, H]
    score = softmax(tanh(q[:,None]+lk) @ v + bv)     [B, S]   (bv drops: softmax shift-invariant)
    context = score @ key                            [B, D]
Outputs (context, score).

Sharding: batch B=32 split 4-per-core across 8 NeuronCores; weights replicated.

v2 design (fp8 DoubleRow):
  - host quantizes key -> fp8e4 (RNE, == TRN fp8_exp4 for |x|<240) plus an
    fp8 residual (key - fp8(key)) used to rebuild ~bf16-accurate key for the
    context matmul. W2*64 and v*64 are host-quantized to fp8 (scale avoids
    subnormals; tanh/exp activations compensate with scale=1/64).
  - per s-tile: key8 natural tile -> PE transpose of bf16-viewed fp8 PAIRS
    (bit-exact, probe-validated) -> kt packed [d-pair, s] -> lin2 via fp8
    DoubleRow matmuls (K=256/instr, 0.5 cyc/row) -> tanh (fused bias q+b1+b2,
    fp8 out) -> score via DoubleRow with v8 -> exp (no max shift needed:
    |score| <~ 2, exp cannot overflow) -> probs transposed to columns ->
    context accumulated in PSUM via tiny stationary-key matmuls
    (fp8 hi + fp8 lo residual) x bf16 prob column.
  - softmax normalization applied at batch end (1/sum via DVE reciprocal,
    partition-broadcast for the context scale).
  - software pipeline: transposes run one tile ahead; exp-transpose + context
    of tile t run in the middle of tile t+1's lin2 to hide latencies.
"""

import numpy as np

import concourse.bacc as bacc
import concourse.mybir as mybir
from concourse.tile import TileContext
from concourse.masks import make_identity

f32 = mybir.dt.float32
bf16 = mybir.dt.bfloat16
fp8 = mybir.dt.float8e4
P = 128
DR = mybir.MatmulPerfMode.DoubleRow
Tanh = mybir.ActivationFunctionType.Tanh
Exp = mybir.ActivationFunctionType.Exp
Copy = mybir.ActivationFunctionType.Copy
Identity = mybir.ActivationFunctionType.Identity
AxX = mybir.AxisListType.X
Add = mybir.AluOpType.add

B_L, S, D, H, Q = 4, 2048, 2048, 1024, 1024
S_T = 256                 # s-tile size
SC = S_T // P             # 2 s-chunks per tile
ST_N = S // S_T           # 8 s-tiles per batch
NT = B_L * ST_N           # 32 tiles total
DC2 = D // 256            # 8 contraction chunks (256 d each) for DoubleRow
HC = H // P               # 8 h-blocks
QC = Q // P               # 8 q-chunks
DQ = D // P               # 16 d-blocks for context
VSCALE = 64.0             # host scale on W2 and v (fp8 subnormal avoidance)


def build_nc():
    nc = bacc.Bacc(None, target_bir_lowering=False)
    key8 = nc.dram_tensor("key8", [B_L, S, D], fp8, kind="ExternalInput")
    key8lo = nc.dram_tensor("key8lo", [B_L, S, D], fp8, kind="ExternalInput")
    w28 = nc.dram_tensor("w28", [P, DC2, 2, H], fp8, kind="ExternalInput")
    w1b = nc.dram_tensor("w1b", [P, QC, H], bf16, kind="ExternalInput")
    qt16 = nc.dram_tensor("qt16", [P, QC, B_L], bf16, kind="ExternalInput")
    b12r = nc.dram_tensor("b12r", [P, HC], f32, kind="ExternalInput")
    vs8 = nc.dram_tensor("vs8", [P, HC // 2, 2, 16], fp8, kind="ExternalInput")
    ctx_out = nc.dram_tensor("ctx_out", [B_L, D], f32, kind="ExternalOutput")
    score_out = nc.dram_tensor("score_out", [B_L, S], f32, kind="ExternalOutput")

    with TileContext(nc) as tc:
        with (
            tc.tile_pool(name="const", bufs=1) as const_pool,
            tc.tile_pool(name="k8p", bufs=4) as k8_pool,
            tc.tile_pool(name="k8lop", bufs=3) as k8lo_pool,
            tc.tile_pool(name="ktp", bufs=2) as kt_pool,
            tc.tile_pool(name="thp", bufs=2) as th_pool,
            tc.tile_pool(name="pcp", bufs=3) as pc_pool,
            tc.tile_pool(name="smp", bufs=1) as sm_pool,
            tc.tile_pool(name="ps", bufs=1, space="PSUM") as ps,
        ):
            # ---------------- prologue ----------------
            ident = const_pool.tile([P, P], f32)
            make_identity(nc, ident)
            ident16 = const_pool.tile([P, P], bf16)
            nc.vector.tensor_copy(ident16[:], ident[:])

            # small weights first: bias path has the longest dependency chain
            w1s = const_pool.tile([P, QC, H], bf16)
            nc.sync.dma_start(w1s[:], w1b[:, :, :])
            qts = const_pool.tile([P, QC, B_L], bf16)
            nc.sync.dma_start(qts[:], qt16[:, :, :])
            b12s = const_pool.tile([P, HC], f32)
            nc.sync.dma_start(b12s[:], b12r[:, :])
            vss = const_pool.tile([P, HC // 2, 2, 16], fp8)
            nc.sync.dma_start(vss[:], vs8[:, :, :, :])

            # first key tiles ahead of the big weight DMA
            def load_k8(t, pool, src):
                b, st = divmod(t, ST_N)
                tile = pool.tile([P, SC, D], fp8, name=pool.name + "t")
                nc.sync.dma_start(
                    tile[:],
                    src[b, st * S_T:(st + 1) * S_T, :].rearrange(
                        "(sc p) d -> p sc d", p=P
                    ),
                )
                return tile

            k8_t = {0: load_k8(0, k8_pool, key8)}

            w28s = const_pool.tile([P, DC2, 2, H], fp8)
            nc.sync.dma_start(w28s[:], w28[:, :, :, :])

            k8_t[1] = load_k8(1, k8_pool, key8)
            k8lo_t = {0: load_k8(0, k8lo_pool, key8lo)}

            # bias_sb[h, m, b] = (query @ W1)[b, h] + b1[h] + b2[h]
            q_ps = ps.tile([P, HC, B_L], f32, name="q_ps", tag="scpt", bufs=2)
            for m in range(HC):
                for k in range(QC):
                    nc.tensor.matmul(
                        q_ps[:, m, :],
                        w1s[:, k, m * P:(m + 1) * P],
                        qts[:, k, :],
                        start=(k == 0),
                        stop=(k == QC - 1),
                        skip_group_check=True,
                    )
            bias_sb = const_pool.tile([P, HC, B_L], f32)
            for m in range(HC):
                nc.scalar.activation(
                    bias_sb[:, m, :], q_ps[:, m, :], Identity,
                    bias=b12s[:, m:m + 1],
                )

            exp4 = const_pool.tile([4, S_T], f32)
            nc.vector.memset(exp4[:], 0.0)
            # zero operands for the ctx-bank clearing matmul (PSUM start=True
            # marks the whole 2KB bank pending-zero, so the bank must be
            # cleared by ONE matmul covering all columns, never per-column)
            zc16 = const_pool.tile([1, P], bf16)
            nc.vector.memset(zc16[:], 0.0)
            zm16 = const_pool.tile([1, DQ], bf16)
            nc.vector.memset(zm16[:], 0.0)

            # ---------------- pipelined main loop ----------------
            # body(t): dma(t+2); transposes(t+1); lin2(t) m0..3; [exp4tp+ctx](t-1);
            #          lin2(t) m4..7; score(t); exp(t)
            kt_t = {}
            pcol_t = {}
            th_t = {}
            exp_row_b = {}
            sums_b = {}
            ctxps_b = {}

            def transposes(t):
                k16v = k8_t[t][:].bitcast(bf16)  # [P, SC, D//2]
                kt = kt_pool.tile([P, DC2, SC, P], bf16, name="kt")
                for half in range(2):
                    tp16 = ps.tile([P, DC2 // 2, SC, P], bf16, name="tp16", bufs=2)
                    for fq in range(DC2 // 2):
                        fb = half * (DC2 // 2) + fq
                        for sc in range(SC):
                            nc.tensor.transpose(
                                tp16[:, fq, sc, :],
                                k16v[:, sc, fb * P:(fb + 1) * P],
                                ident16[:],
                            )
                    nc.vector.tensor_copy(
                        kt[:, half * (DC2 // 2):(half + 1) * (DC2 // 2)], tp16[:]
                    )
                kt_t[t] = kt

            def lin2(t, m_lo, m_hi):
                b = t // ST_N
                kt = kt_t[t]
                if m_lo == 0:
                    th_t[t] = th_pool.tile([P, HC, S_T], fp8, name="th")
                th = th_t[t]
                for m in range(m_lo, m_hi):
                    lk = ps.tile([P, S_T], f32, name="lk", bufs=3)
                    for c in range(DC2):
                        mv = kt[:, c].bitcast(fp8).rearrange(
                            "p sc (s two) -> p two sc s", two=2
                        )
                        nc.tensor.matmul(
                            lk[:],
                            w28s[:, c, :, m * P:(m + 1) * P],
                            mv,
                            start=(c == 0),
                            stop=(c == DC2 - 1),
                            perf_mode=DR,
                            skip_group_check=True,
                        )
                    nc.scalar.activation(
                        th[:, m, :], lk[:], Tanh,
                        bias=bias_sb[:, m, b:b + 1], scale=1.0 / VSCALE,
                    )

            def score_exp(t):
                b, st = divmod(t, ST_N)
                th = th_t[t]
                sc_ps = ps.tile([1, S_T], f32, name="sc_ps", tag="scpt", bufs=2)
                for m2 in range(HC // 2):
                    nc.tensor.matmul(
                        sc_ps[:],
                        vss[:, m2, :, 0:1],
                        th[:, 2 * m2:2 * m2 + 2, :],
                        start=(m2 == 0),
                        stop=(m2 == HC // 2 - 1),
                        perf_mode=DR,
                        skip_group_check=True,
                    )
                if st == 0:
                    exp_row_b[b] = sm_pool.tile([1, S], f32, name="exp_row", bufs=2)
                    sums_b[b] = sm_pool.tile([1, ST_N], f32, name="sums", bufs=2)
                nc.scalar.activation(
                    exp4[0:1, :], sc_ps[:], Exp, scale=1.0 / VSCALE,
                    accum_out=sums_b[b][0:1, st:st + 1],
                )
                nc.vector.tensor_copy(
                    exp_row_b[b][0:1, st * S_T:(st + 1) * S_T], exp4[0:1, :]
                )

            def exp_cols_ctx(t):
                b, st = divmod(t, ST_N)
                # probs columns (bf16) from exp4 row
                pcol = pc_pool.tile([P, SC], bf16, name="pcol")
                for sc in range(SC):
                    ptp = ps.tile([P, 4], f32, name="ptp", tag="scpt", bufs=2)
                    nc.tensor.transpose(
                        ptp[:], exp4[:, sc * P:(sc + 1) * P], ident[:4, :4]
                    )
                    nc.vector.tensor_copy(pcol[:, sc:sc + 1], ptp[:, 0:1])
                pcol_t[t] = pcol
                # context accumulation: stationary key blocks x prob column
                if st == 0:
                    ctxps_b[b] = ps.tile([P, DQ], f32, name="ctx_ps", bufs=1)
                    nc.tensor.matmul(
                        ctxps_b[b][:], zc16[:], zm16[:],
                        start=True, stop=False, skip_group_check=True,
                    )
                ctx_ps = ctxps_b[b]
                for sc in range(SC):
                    for dq in range(DQ):
                        nc.tensor.matmul(
                            ctx_ps[:, dq:dq + 1],
                            k8_t[t][:, sc, dq * P:(dq + 1) * P],
                            pcol[:, sc:sc + 1],
                            start=False,
                            stop=False,
                            skip_group_check=True,
                        )
                        nc.tensor.matmul(
                            ctx_ps[:, dq:dq + 1],
                            k8lo_t[t][:, sc, dq * P:(dq + 1) * P],
                            pcol[:, sc:sc + 1],
                            start=False,
                            stop=(st == ST_N - 1 and sc == SC - 1 and dq == DQ - 1),
                            skip_group_check=True,
                        )
                del k8_t[t], k8lo_t[t], pcol_t[t]

            def finalize(b):
                tot = sm_pool.tile([1, 2], f32, name="tot", bufs=2)
                nc.vector.tensor_reduce(
                    tot[0:1, 0:1], sums_b[b][:], axis=AxX, op=Add
                )
                nc.vector.reciprocal(tot[0:1, 1:2], tot[0:1, 0:1])
                inv = tot[0:1, 1:2]
                erow = exp_row_b[b]
                nc.vector.tensor_scalar_mul(erow[:], erow[:], inv)
                nc.sync.dma_start(score_out[b:b + 1, :], erow[:])
                inv128 = sm_pool.tile([P, 1], f32, name="inv128", bufs=2)
                nc.gpsimd.partition_broadcast(inv128[:], inv)
                ctx_sb = sm_pool.tile([P, DQ], f32, name="ctx_sb", bufs=2)
                nc.scalar.activation(
                    ctx_sb[:], ctxps_b[b][:], Copy, scale=inv128[:, 0:1]
                )
                crow_ps = ps.tile([DQ, P], f32, name="crow", tag="scpt", bufs=2)
                nc.tensor.transpose(crow_ps[:], ctx_sb[:], ident[:])
                crow = sm_pool.tile([DQ, P], f32, name="crow_sb", bufs=2)
                nc.vector.tensor_copy(crow[:], crow_ps[:])
                nc.sync.dma_start(
                    ctx_out[b].rearrange("(dq p) -> dq p", p=P), crow[:]
                )
                del exp_row_b[b], sums_b[b], ctxps_b[b]

            for t in range(NT + 1):
                if t + 2 <= NT - 1:
                    k8_t[t + 2] = load_k8(t + 2, k8_pool, key8)
                if t + 1 <= NT - 1:
                    k8lo_t[t + 1] = load_k8(t + 1, k8lo_pool, key8lo)
                if t == 0:
                    transposes(0)
                if t + 1 <= NT - 1:
                    transposes(t + 1)
                if t <= NT - 1:
                    lin2(t, 0, HC // 2)
                if t >= 1:
                    exp_cols_ctx(t - 1)
                    if t % ST_N == 0:
                        finalize(t // ST_N - 1)
                if t <= NT - 1:
                    lin2(t, HC // 2, HC)
                    score_exp(t)

    nc.compile()
    return nc


_NC_CACHE = {}


def _get_nc():
    if "full" not in _NC_CACHE:
        _NC_CACHE["full"] = build_nc()
    return _NC_CACHE["full"]


def kernel(**inputs):
    import ml_dtypes
    from concourse.bass_utils import run_bass_kernel_spmd

    E4 = ml_dtypes.float8_e4m3
    BF = ml_dtypes.bfloat16

    query = np.ascontiguousarray(inputs["query"], dtype=np.float32)
    key = np.ascontiguousarray(inputs["key"], dtype=np.float32)
    W1 = np.ascontiguousarray(inputs["W1"], dtype=np.float32)
    b1 = np.ascontiguousarray(inputs["b1"], dtype=np.float32)
    W2 = np.ascontiguousarray(inputs["W2"], dtype=np.float32)
    b2 = np.ascontiguousarray(inputs["b2"], dtype=np.float32)
    v = np.ascontiguousarray(inputs["v"], dtype=np.float32)
    # bv is softmax-shift-invariant: it cancels in both outputs.

    B = query.shape[0]
    n_cores = 8
    bl = B // n_cores

    # host-side quantization (RNE; ml_dtypes float8_e4m3 == TRN fp8_exp4
    # bit-for-bit in our value range)
    key8 = key.astype(E4)
    key8lo = (key - key8.astype(np.float32)).astype(E4)
    w28 = (W2 * VSCALE).astype(np.float32).reshape(DC2, P, 2, H) \
        .transpose(1, 0, 2, 3).astype(E4)
    w28 = np.ascontiguousarray(w28)
    w1b = np.ascontiguousarray(W1.reshape(QC, P, H).transpose(1, 0, 2)).astype(BF)
    b12 = np.ascontiguousarray((b1 + b2).reshape(HC, P).T, dtype=np.float32)
    vs = np.zeros((P, HC // 2, 2, 16), dtype=E4)
    vs[:, :, :, 0] = (v * VSCALE).reshape(HC // 2, 2, P).transpose(2, 0, 1).astype(E4)

    nc = _get_nc()
    in_maps = []
    for c in range(n_cores):
        sl = slice(c * bl, (c + 1) * bl)
        qt = np.ascontiguousarray(
            query[sl].reshape(bl, QC, P).transpose(2, 1, 0)
        ).astype(BF)
        in_maps.append({
            "key8": key8[sl], "key8lo": key8lo[sl],
            "w28": w28, "w1b": w1b, "qt16": qt, "b12r": b12, "vs8": vs,
        })
    res = run_bass_kernel_spmd(nc, in_maps, core_ids=list(range(n_cores)))
    context = np.concatenate([r["ctx_out"] for r in res.results], axis=0)
    score = np.concatenate([r["score_out"] for r in res.results], axis=0)
    return (context.astype(np.float32), score.astype(np.float32))
